# revision 1
# baseline (speedup 1.0000x reference)
"""Tricubic B-spline grid interpolation (CubicBSplineGrid3d) on 8 Trainium2 cores.

Strategy (data-parallel over queries, per sharding hint):
  * Host: pad grid (64,64,64,32) -> (67,67,67,32) edge-replicated, then pack the
    4x4 (d,h)-neighborhood redundantly:
        Q[d, h, w, i, j, c] = Gpad[d+i, h+j, w, c]   (f32, ~2.25 GB)
    so each query's full 4x4x4x32 neighborhood is ONE 8KB contiguous run
    Q.flat[base*2048 : base*2048+2048], base=(sd*64+sh)*67+sw.  Exactly the
    information-minimal 8KB/query is fetched, in one 8KB descriptor/partition.
  * Device (per core, 16384 queries = 128 blocks of 128):
      stage 1: compute floor/frac of u*63, cubic basis weights, gather base
               indices; PE-transpose block-layout -> query-on-partition layout.
      main loop: one indirect DMA gathers a [128, 2048] tile (query on
               partition); 16 fused mul-add DVE ops contract (d,h) with
               per-partition scalars; 4 ops contract w; DMA out [128, 32].
  * Host: concat the 8 cores' outputs.
"""
import sys

for _p in ("/opt/trn_rl_repo",):
    if _p not in sys.path:
        sys.path.insert(0, _p)

import numpy as np

N_CORES = 8
B_GLOBAL = 131072
B_LOCAL = B_GLOBAL // N_CORES          # 16384
NBLK = B_LOCAL // 128                  # 128 blocks of 128 queries
GD = GH = GW = 64                      # grid spatial dims
GC = 32                                # channels
QROWS = GD * GH * (GW + 3)             # 64*64*67 = 274432
QROWLEN = 4 * 4 * GC                   # 512 elements = 2KB per (d,h,w) row
GLEN = 4 * QROWLEN                     # 2048 elements = 8KB gathered (4 w-rows)

_nc_cache = None


def _build_nc():
    """Build + compile the per-core Bass program (identical on all cores)."""
    from concourse import bacc, mybir
    from concourse.bass import IndirectOffsetOnAxis
    from concourse.tile import TileContext
    from concourse.masks import make_identity

    f32, i32 = mybir.dt.float32, mybir.dt.int32
    Alu = mybir.AluOpType
    Act = mybir.ActivationFunctionType
    P = 128

    nc = bacc.Bacc("TRN2", target_bir_lowering=False, debug=False,
                   num_devices=N_CORES)
    u_t = nc.dram_tensor("u", [B_LOCAL, 3], f32, kind="ExternalInput")
    q_t = nc.dram_tensor("q", [QROWS, QROWLEN], f32, kind="ExternalInput")
    o_t = nc.dram_tensor("o", [B_LOCAL, GC], f32, kind="ExternalOutput")

    with TileContext(nc) as tc:
        with (
            tc.tile_pool(name="persist", bufs=1) as pp,
            tc.tile_pool(name="stage1", bufs=1) as s1,
            tc.tile_pool(name="psum", bufs=2, space="PSUM") as psum,
            tc.tile_pool(name="g", bufs=6) as gp,
            tc.tile_pool(name="acc", bufs=3) as ap_,
            tc.tile_pool(name="o", bufs=4) as op_,
        ):
            # ---------- stage 1: weights + indices (block layout) ----------
            # U[p, n, a] = u[p*128 + n, a]; per-partition 1536B contiguous.
            U = s1.tile([P, 384], f32)
            nc.sync.dma_start(
                out=U[:, :], in_=u_t[:, :].rearrange("(p n) c -> p (n c)", p=P))
            X = s1.tile([P, 384], f32)
            nc.vector.tensor_scalar(X[:, :], U[:, :], float(GD - 1), None, Alu.mult)
            # floor via round-to-nearest cast + correction
            Si = s1.tile([P, 384], i32)
            nc.vector.tensor_copy(out=Si[:, :], in_=X[:, :])
            Sf = s1.tile([P, 384], f32)
            nc.vector.tensor_copy(out=Sf[:, :], in_=Si[:, :])
            D = s1.tile([P, 384], f32)
            nc.vector.tensor_tensor(out=D[:, :], in0=X[:, :], in1=Sf[:, :],
                                    op=Alu.subtract)
            M = s1.tile([P, 384], f32)
            nc.vector.tensor_scalar(M[:, :], D[:, :], 0.0, None, Alu.is_lt)
            S = s1.tile([P, 384], f32)
            nc.vector.tensor_tensor(out=S[:, :], in0=Sf[:, :], in1=M[:, :],
                                    op=Alu.subtract)
            T = s1.tile([P, 384], f32)
            nc.vector.tensor_tensor(out=T[:, :], in0=X[:, :], in1=S[:, :],
                                    op=Alu.subtract)

            S3 = S[:, :].rearrange("p (n c) -> p n c", c=3)
            # base = (sd*64 + sh)*67 + sw
            Bse = s1.tile([P, 128], f32)
            nc.vector.scalar_tensor_tensor(
                out=Bse[:, :], in0=S3[:, :, 0], scalar=float(GH),
                in1=S3[:, :, 1], op0=Alu.mult, op1=Alu.add)
            nc.vector.scalar_tensor_tensor(
                out=Bse[:, :], in0=Bse[:, :], scalar=float(GW + 3),
                in1=S3[:, :, 2], op0=Alu.mult, op1=Alu.add)

            # cubic basis weights on [128, 384] (all 3 axes at once)
            T2 = s1.tile([P, 384], f32)
            nc.vector.tensor_tensor(out=T2[:, :], in0=T[:, :], in1=T[:, :],
                                    op=Alu.mult)
            T3 = s1.tile([P, 384], f32)
            nc.vector.tensor_tensor(out=T3[:, :], in0=T2[:, :], in1=T[:, :],
                                    op=Alu.mult)
            sixth = 1.0 / 6.0
            W0 = s1.tile([P, 384], f32)
            nc.vector.tensor_scalar(W0[:, :], T3[:, :], -sixth, None, Alu.mult)
            nc.vector.scalar_tensor_tensor(out=W0[:, :], in0=T2[:, :], scalar=0.5,
                                           in1=W0[:, :], op0=Alu.mult, op1=Alu.add)
            nc.vector.scalar_tensor_tensor(out=W0[:, :], in0=T[:, :], scalar=-0.5,
                                           in1=W0[:, :], op0=Alu.mult, op1=Alu.add)
            nc.vector.tensor_scalar(W0[:, :], W0[:, :], sixth, None, Alu.add)
            W1 = s1.tile([P, 384], f32)
            nc.vector.tensor_scalar(W1[:, :], T3[:, :], 0.5, None, Alu.mult)
            nc.vector.scalar_tensor_tensor(out=W1[:, :], in0=T2[:, :], scalar=-1.0,
                                           in1=W1[:, :], op0=Alu.mult, op1=Alu.add)
            nc.vector.tensor_scalar(W1[:, :], W1[:, :], 2.0 / 3.0, None, Alu.add)
            W3 = s1.tile([P, 384], f32)
            nc.vector.tensor_scalar(W3[:, :], T3[:, :], sixth, None, Alu.mult)
            # w2 = 1 - w0 - w1 - w3  (partition of unity)
            W2 = s1.tile([P, 384], f32)
            nc.vector.tensor_tensor(out=W2[:, :], in0=W0[:, :], in1=W1[:, :],
                                    op=Alu.add)
            nc.vector.tensor_tensor(out=W2[:, :], in0=W2[:, :], in1=W3[:, :],
                                    op=Alu.add)
            nc.vector.tensor_scalar(W2[:, :], W2[:, :], -1.0, 1.0,
                                    Alu.mult, Alu.add)

            # ---------- transposes to query-on-partition layout ----------
            ident = pp.tile([P, P], f32)
            make_identity(nc, ident[:, :])

            TD = pp.tile([P, 512], f32)   # wd_i  at cols i*128 + b
            TH = pp.tile([P, 512], f32)   # wh_j  at cols j*128 + b
            TW = pp.tile([P, 512], f32)   # ww_k  at cols k*128 + b
            FB = pp.tile([P, 128], f32)   # base  [query, block]
            Ws = [W0, W1, W2, W3]

            def transpose_into(dst_ap, src_ap):
                pt = psum.tile([P, P], f32, space="PSUM")
                nc.tensor.transpose(out=pt[:, :], in_=src_ap, identity=ident[:, :])
                nc.vector.tensor_copy(out=dst_ap, in_=pt[:, :])

            for a, Tt in ((0, TD), (1, TH), (2, TW)):
                for i in range(4):
                    w3v = Ws[i][:, :].rearrange("p (n c) -> p n c", c=3)
                    transpose_into(Tt[:, i * 128:(i + 1) * 128], w3v[:, :, a])
            transpose_into(FB[:, :], Bse[:, :])

            IdxI = pp.tile([P, 128], i32)
            nc.vector.tensor_copy(out=IdxI[:, :], in_=FB[:, :])

            # wdh_all[q, (i*4+j)*128 + b] = wd_i[q,b] * wh_j[q,b]
            WDH = pp.tile([P, 2048], f32)
            for i in range(4):
                for j in range(4):
                    nc.vector.tensor_tensor(
                        out=WDH[:, (i * 4 + j) * 128:(i * 4 + j + 1) * 128],
                        in0=TD[:, i * 128:(i + 1) * 128],
                        in1=TH[:, j * 128:(j + 1) * 128],
                        op=Alu.mult)

            # ---------- main loop over 128 query blocks ----------
            # G run layout per partition: [w(4), c(32), ij(16)] contiguous.
            # pass A: A4[blk] = sum_k ww_k * G[:, k*512:(k+1)*512]   (w contract)
            # pass B (batched over 4 blocks): o = reduce_ij(A4 * wdh) (d,h)
            WDHv = WDH[:, :].rearrange("p (ij b) -> p b ij", b=128)
            for b in range(NBLK):
                blk = b % 4
                G = gp.tile([P, GLEN], f32)
                nc.gpsimd.indirect_dma_start(
                    out=G[:, :],
                    out_offset=None,
                    in_=q_t[:, :],
                    in_offset=IndirectOffsetOnAxis(ap=IdxI[:, b:b + 1], axis=0),
                )
                if blk == 0:
                    A4 = ap_.tile([P, 4 * QROWLEN], f32)
                Asl = A4[:, blk * QROWLEN:(blk + 1) * QROWLEN]
                nc.vector.tensor_scalar(Asl, G[:, 0:QROWLEN],
                                        TW[:, b:b + 1], None, Alu.mult)
                for k in range(1, 4):
                    nc.vector.scalar_tensor_tensor(
                        out=Asl, in0=G[:, k * QROWLEN:(k + 1) * QROWLEN],
                        scalar=TW[:, k * 128 + b:k * 128 + b + 1],
                        in1=Asl, op0=Alu.mult, op1=Alu.add)
                if blk == 3:
                    b0 = b - 3
                    A4v = A4[:, :].rearrange("p (blk c ij) -> p blk c ij",
                                             blk=4, ij=16)
                    wb = (WDHv[:, b0:b0 + 4, :]
                          .rearrange("p blk (x ij) -> p blk x ij", x=1)
                          .to_broadcast([P, 4, GC, 16]))
                    Pm = ap_.tile([P, 4 * QROWLEN], f32)
                    Pm4 = Pm[:, :].rearrange("p (blk c ij) -> p blk c ij",
                                             blk=4, ij=16)
                    nc.vector.tensor_tensor(out=Pm4[:, :, :, :],
                                            in0=A4v[:, :, :, :],
                                            in1=wb, op=Alu.mult)
                    o4 = op_.tile([P, 4, GC], f32)
                    nc.vector.tensor_reduce(
                        out=o4[:, :, :], in_=Pm4[:, :, :, :],
                        axis=mybir.AxisListType.X, op=Alu.add)
                    nc.sync.dma_start(
                        out=o_t[b0 * 128:(b + 1) * 128, :].rearrange(
                            "(blk q) c -> q blk c", blk=4),
                        in_=o4[:, :, :])
    nc.compile()
    return nc


def _pack_grid(grid: np.ndarray) -> np.ndarray:
    """(64,64,64,32) -> [QROWS, QROWLEN] f32 with
    Q[d,h,w,i,j,c] = Gpad[d+i, h+j, w, c]."""
    gp = np.pad(grid, ((1, 2), (1, 2), (1, 2), (0, 0)), mode="edge")
    win = np.lib.stride_tricks.sliding_window_view(gp, (4, 4), axis=(0, 1))
    # win: [64, 64, 67, 32, 4, 4] = (d, h, w, c, i, j); keep ij innermost so
    # the on-device (d,h) contraction can use tensor_reduce over X.
    q = np.ascontiguousarray(win, dtype=np.float32)
    return q.reshape(QROWS, QROWLEN)


def kernel(u: np.ndarray, grid: np.ndarray) -> np.ndarray:
    global _nc_cache
    from concourse.bass_utils import run_bass_kernel_spmd

    assert u.shape == (B_GLOBAL, 3) and grid.shape == (GD, GH, GW, GC)
    if _nc_cache is None:
        _nc_cache = _build_nc()
    nc = _nc_cache

    q = _pack_grid(np.asarray(grid, dtype=np.float32))
    u = np.ascontiguousarray(u, dtype=np.float32)
    in_maps = [
        {"u": u[c * B_LOCAL:(c + 1) * B_LOCAL], "q": q} for c in range(N_CORES)
    ]
    res = run_bass_kernel_spmd(nc, in_maps, core_ids=list(range(N_CORES)))
    out = np.concatenate([res.results[c]["o"] for c in range(N_CORES)], axis=0)
    return out.astype(np.float32)


if __name__ == "__main__":
    # quick self-run with random inputs
    rng = np.random.default_rng(0)
    grid = rng.standard_normal((GD, GH, GW, GC), dtype=np.float32)
    u = rng.random((B_GLOBAL, 3), dtype=np.float32)
    out = kernel(u, grid)
    print("out", out.shape, out.dtype, float(np.abs(out).mean()))



# revision 3
# speedup vs baseline: 1.2467x; 1.2467x over previous
"""Tricubic B-spline grid interpolation (CubicBSplineGrid3d) on 8 Trainium2 cores.

Strategy (data-parallel over queries, per sharding hint):
  * Host: fold the w-axis B-spline basis matrix into the packed grid
    (polynomial basis): rows keyed (d, h, sw) hold
        Q[d, h, sw, m, c, i, j] = sum_k M[m,k] Gpad[d+i, h+j, sw+k, c]
    in bf16, so each query's data is ONE contiguous 4KB run and the
    on-device w-contraction is a 3-op Horner chain in t_w.
  * Device (per core, 16384 queries = 128 blocks of 128):
      stage 1: floor/frac of u*63, cubic basis weights for d,h axes,
               raw t_w, base indices; PE-transpose to query-on-partition.
      main loop: indirect DMA gathers [128, 2048] bf16 (query on
               partition); 3 fused mul-add (Horner in t_w); per 4 blocks:
               multiply by wd_i*wh_j (ij innermost, 2x mode) and add-tree
               over ij; DMA out [128, 4, 32] f32.
  * Host: concat the 8 cores' outputs.
"""
import sys

for _p in ("/opt/trn_rl_repo",):
    if _p not in sys.path:
        sys.path.insert(0, _p)

import numpy as np

N_CORES = 8
B_GLOBAL = 131072
B_LOCAL = B_GLOBAL // N_CORES          # 16384
NBLK = B_LOCAL // 128                  # 128 blocks of 128 queries
GD = GH = GW = 64                      # grid spatial dims
GC = 32                                # channels
QROWS = GD * GH * GW                   # 64*64*64 = 262144
GLEN = 4 * 4 * 4 * GC                  # 2048 elements = 4KB bf16 per query
MSLICE = 4 * 4 * GC                    # 512 elements per Horner coefficient

_nc_cache = None


def _build_nc():
    """Build + compile the per-core Bass program (identical on all cores)."""
    from concourse import bacc, mybir
    from concourse.bass import IndirectOffsetOnAxis
    from concourse.tile import TileContext
    from concourse.masks import make_identity

    f32, i32 = mybir.dt.float32, mybir.dt.int32
    bf16 = mybir.dt.bfloat16
    Alu = mybir.AluOpType
    P = 128

    nc = bacc.Bacc("TRN2", target_bir_lowering=False, debug=False,
                   num_devices=N_CORES)
    u_t = nc.dram_tensor("u", [B_LOCAL, 3], f32, kind="ExternalInput")
    q_t = nc.dram_tensor("q", [QROWS, GLEN], bf16, kind="ExternalInput")
    o_t = nc.dram_tensor("o", [B_LOCAL, GC], f32, kind="ExternalOutput")

    with TileContext(nc) as tc:
        with (
            tc.tile_pool(name="persist", bufs=1) as pp,
            tc.tile_pool(name="stage1", bufs=1) as s1,
            tc.tile_pool(name="psum", bufs=2, space="PSUM") as psum,
            tc.tile_pool(name="g", bufs=6) as gp,
            tc.tile_pool(name="x", bufs=3) as xp,
            tc.tile_pool(name="acc", bufs=2) as ap_,
            tc.tile_pool(name="tree", bufs=2) as tp_,
            tc.tile_pool(name="o", bufs=4) as op_,
        ):
            # ---------- stage 1: weights + indices (block layout) ----------
            # U[p, n, a] = u[p*128 + n, a]; per-partition 1536B contiguous.
            U = s1.tile([P, 384], f32)
            nc.sync.dma_start(
                out=U[:, :], in_=u_t[:, :].rearrange("(p n) c -> p (n c)", p=P))
            X = s1.tile([P, 384], f32)
            nc.vector.tensor_scalar(X[:, :], U[:, :], float(GD - 1), None, Alu.mult)
            # floor via round-to-nearest cast + correction
            Si = s1.tile([P, 384], i32)
            nc.vector.tensor_copy(out=Si[:, :], in_=X[:, :])
            Sf = s1.tile([P, 384], f32)
            nc.vector.tensor_copy(out=Sf[:, :], in_=Si[:, :])
            D = s1.tile([P, 384], f32)
            nc.vector.tensor_tensor(out=D[:, :], in0=X[:, :], in1=Sf[:, :],
                                    op=Alu.subtract)
            M = s1.tile([P, 384], f32)
            nc.vector.tensor_scalar(M[:, :], D[:, :], 0.0, None, Alu.is_lt)
            S = s1.tile([P, 384], f32)
            nc.vector.tensor_tensor(out=S[:, :], in0=Sf[:, :], in1=M[:, :],
                                    op=Alu.subtract)
            T = s1.tile([P, 384], f32)
            nc.vector.tensor_tensor(out=T[:, :], in0=X[:, :], in1=S[:, :],
                                    op=Alu.subtract)

            S3 = S[:, :].rearrange("p (n c) -> p n c", c=3)
            # base = (sd*64 + sh)*64 + sw
            Bse = s1.tile([P, 128], f32)
            nc.vector.scalar_tensor_tensor(
                out=Bse[:, :], in0=S3[:, :, 0], scalar=float(GH),
                in1=S3[:, :, 1], op0=Alu.mult, op1=Alu.add)
            nc.vector.scalar_tensor_tensor(
                out=Bse[:, :], in0=Bse[:, :], scalar=float(GW),
                in1=S3[:, :, 2], op0=Alu.mult, op1=Alu.add)

            # cubic basis weights on [128, 384] (d,h cols used; w discarded)
            T2 = s1.tile([P, 384], f32)
            nc.vector.tensor_tensor(out=T2[:, :], in0=T[:, :], in1=T[:, :],
                                    op=Alu.mult)
            T3 = s1.tile([P, 384], f32)
            nc.vector.tensor_tensor(out=T3[:, :], in0=T2[:, :], in1=T[:, :],
                                    op=Alu.mult)
            sixth = 1.0 / 6.0
            W0 = s1.tile([P, 384], f32)
            nc.vector.tensor_scalar(W0[:, :], T3[:, :], -sixth, None, Alu.mult)
            nc.vector.scalar_tensor_tensor(out=W0[:, :], in0=T2[:, :], scalar=0.5,
                                           in1=W0[:, :], op0=Alu.mult, op1=Alu.add)
            nc.vector.scalar_tensor_tensor(out=W0[:, :], in0=T[:, :], scalar=-0.5,
                                           in1=W0[:, :], op0=Alu.mult, op1=Alu.add)
            nc.vector.tensor_scalar(W0[:, :], W0[:, :], sixth, None, Alu.add)
            W1 = s1.tile([P, 384], f32)
            nc.vector.tensor_scalar(W1[:, :], T3[:, :], 0.5, None, Alu.mult)
            nc.vector.scalar_tensor_tensor(out=W1[:, :], in0=T2[:, :], scalar=-1.0,
                                           in1=W1[:, :], op0=Alu.mult, op1=Alu.add)
            nc.vector.tensor_scalar(W1[:, :], W1[:, :], 2.0 / 3.0, None, Alu.add)
            W3 = s1.tile([P, 384], f32)
            nc.vector.tensor_scalar(W3[:, :], T3[:, :], sixth, None, Alu.mult)
            # w2 = 1 - w0 - w1 - w3  (partition of unity)
            W2 = s1.tile([P, 384], f32)
            nc.vector.tensor_tensor(out=W2[:, :], in0=W0[:, :], in1=W1[:, :],
                                    op=Alu.add)
            nc.vector.tensor_tensor(out=W2[:, :], in0=W2[:, :], in1=W3[:, :],
                                    op=Alu.add)
            nc.vector.tensor_scalar(W2[:, :], W2[:, :], -1.0, 1.0,
                                    Alu.mult, Alu.add)

            # ---------- transposes to query-on-partition layout ----------
            ident = pp.tile([P, P], f32)
            make_identity(nc, ident[:, :])

            TD = pp.tile([P, 512], f32)   # wd_i  at cols i*128 + b
            TH = pp.tile([P, 512], f32)   # wh_j  at cols j*128 + b
            TWt = pp.tile([P, 128], f32)  # raw t_w per (query, block)
            FB = pp.tile([P, 128], f32)   # base  [query, block]
            Ws = [W0, W1, W2, W3]

            def transpose_into(dst_ap, src_ap):
                pt = psum.tile([P, P], f32, space="PSUM")
                nc.tensor.transpose(out=pt[:, :], in_=src_ap, identity=ident[:, :])
                nc.vector.tensor_copy(out=dst_ap, in_=pt[:, :])

            for a, Tt in ((0, TD), (1, TH)):
                for i in range(4):
                    w3v = Ws[i][:, :].rearrange("p (n c) -> p n c", c=3)
                    transpose_into(Tt[:, i * 128:(i + 1) * 128], w3v[:, :, a])
            Tv = T[:, :].rearrange("p (n c) -> p n c", c=3)
            transpose_into(TWt[:, :], Tv[:, :, 2])
            transpose_into(FB[:, :], Bse[:, :])

            IdxI = pp.tile([P, 128], i32)
            nc.vector.tensor_copy(out=IdxI[:, :], in_=FB[:, :])

            # WDH2[q, b*16 + (i*4+j)] = wd_i[q,b] * wh_j[q,b]  (ij inner)
            WDH2 = pp.tile([P, 2048], f32)
            W2v = WDH2[:, :].rearrange("p (b ij) -> p b ij", ij=16)
            for i in range(4):
                for j in range(4):
                    nc.vector.tensor_tensor(
                        out=W2v[:, :, i * 4 + j],
                        in0=TD[:, i * 128:(i + 1) * 128],
                        in1=TH[:, j * 128:(j + 1) * 128],
                        op=Alu.mult)
            WDH2b = pp.tile([P, 2048], bf16)
            nc.vector.tensor_copy(out=WDH2b[:, :], in_=WDH2[:, :])

            # ---------- main loop over 128 query blocks ----------
            # G run layout per partition: [m(4), c(32), ij(16)] bf16.
            # Horner in t_w: A = ((P3*t + P2)*t + P1)*t + P0 -> [c, ij]
            # per 4 blocks: Pm = A4 * wdh (ij inner, 2x); add-tree over ij.
            for b in range(NBLK):
                blk = b % 4
                G = gp.tile([P, GLEN], bf16)
                nc.gpsimd.indirect_dma_start(
                    out=G[:, :],
                    out_offset=None,
                    in_=q_t[:, :],
                    in_offset=IndirectOffsetOnAxis(ap=IdxI[:, b:b + 1], axis=0),
                )
                tw = TWt[:, b:b + 1]
                Xt = xp.tile([P, MSLICE], bf16)
                nc.vector.scalar_tensor_tensor(
                    out=Xt[:, :], in0=G[:, 3 * MSLICE:4 * MSLICE],
                    scalar=tw, in1=G[:, 2 * MSLICE:3 * MSLICE],
                    op0=Alu.mult, op1=Alu.add)
                nc.vector.scalar_tensor_tensor(
                    out=Xt[:, :], in0=Xt[:, :],
                    scalar=tw, in1=G[:, 1 * MSLICE:2 * MSLICE],
                    op0=Alu.mult, op1=Alu.add)
                if blk == 0:
                    A4 = ap_.tile([P, 4 * MSLICE], bf16)
                nc.vector.scalar_tensor_tensor(
                    out=A4[:, blk * MSLICE:(blk + 1) * MSLICE], in0=Xt[:, :],
                    scalar=tw, in1=G[:, 0:MSLICE],
                    op0=Alu.mult, op1=Alu.add)
                if blk == 3:
                    b0 = b - 3
                    A4v = A4[:, :].rearrange("p (blk c ij) -> p blk c ij",
                                             blk=4, ij=16)
                    wb = (WDH2b[:, b0 * 16:(b0 + 4) * 16]
                          .rearrange("p (blk x ij) -> p blk x ij", blk=4, x=1)
                          .to_broadcast([P, 4, GC, 16]))
                    Pm = ap_.tile([P, 4 * MSLICE], bf16)
                    Pm4 = Pm[:, :].rearrange("p (blk c ij) -> p blk c ij",
                                             blk=4, ij=16)
                    nc.vector.tensor_tensor(out=Pm4[:, :, :, :],
                                            in0=A4v[:, :, :, :],
                                            in1=wb, op=Alu.mult)
                    # add-tree over ij (innermost 16 -> 1)
                    T1 = tp_.tile([P, 4, GC, 8], bf16)
                    nc.vector.tensor_tensor(out=T1[:, :, :, :],
                                            in0=Pm4[:, :, :, 0:8],
                                            in1=Pm4[:, :, :, 8:16], op=Alu.add)
                    T2t = tp_.tile([P, 4, GC, 4], bf16)
                    nc.vector.tensor_tensor(out=T2t[:, :, :, :],
                                            in0=T1[:, :, :, 0:4],
                                            in1=T1[:, :, :, 4:8], op=Alu.add)
                    T3t = tp_.tile([P, 4, GC, 2], bf16)
                    nc.vector.tensor_tensor(out=T3t[:, :, :, :],
                                            in0=T2t[:, :, :, 0:2],
                                            in1=T2t[:, :, :, 2:4], op=Alu.add)
                    o4 = op_.tile([P, 4, GC], f32)
                    nc.vector.tensor_tensor(out=o4[:, :, :],
                                            in0=T3t[:, :, :, 0],
                                            in1=T3t[:, :, :, 1], op=Alu.add)
                    nc.sync.dma_start(
                        out=o_t[b0 * 128:(b + 1) * 128, :].rearrange(
                            "(blk q) c -> q blk c", blk=4),
                        in_=o4[:, :, :])
    nc.compile()
    return nc


# Uniform cubic B-spline basis matrix: ww_k(t) = sum_m BMAT[m,k] * t^m
BMAT = np.array([
    [1.0, 4.0, 1.0, 0.0],
    [-3.0, 0.0, 3.0, 0.0],
    [3.0, -6.0, 3.0, 0.0],
    [-1.0, 3.0, -3.0, 1.0],
], dtype=np.float32) / 6.0


def _pack_grid(grid: np.ndarray) -> np.ndarray:
    """(64,64,64,32) -> [QROWS, GLEN] bf16 with
    Q[d,h,sw, m, c, i, j] = sum_k BMAT[m,k] Gpad[d+i, h+j, sw+k, c]."""
    import ml_dtypes

    gp = np.pad(np.asarray(grid, dtype=np.float32),
                ((1, 2), (1, 2), (1, 2), (0, 0)), mode="edge")
    # fold the w-axis basis first on the small padded grid:
    # PM[dp, hp, sw, m, c] = sum_k BMAT[m,k] gp[dp, hp, sw+k, c]
    gpw = np.lib.stride_tricks.sliding_window_view(gp, 4, axis=2)
    # gpw: [67, 67, 64, 32, 4]
    PM = np.einsum("mk,abwck->abwmc", BMAT, gpw, optimize=True)
    # window (i,j): Q[d,h,w, m, c, i, j] = PM[d+i, h+j, w, m, c]
    win = np.lib.stride_tricks.sliding_window_view(PM, (4, 4), axis=(0, 1))
    # win: [64, 64, 64, 4m, 32c, 4i, 4j]
    out = np.empty((QROWS, GLEN), dtype=ml_dtypes.bfloat16)
    ov = out.reshape(GD, GH, GW, 4, GC, 4, 4)
    for d0 in range(0, GD, 8):
        ov[d0:d0 + 8] = win[d0:d0 + 8].astype(ml_dtypes.bfloat16)
    return out


def kernel(u: np.ndarray, grid: np.ndarray) -> np.ndarray:
    global _nc_cache
    from concourse.bass_utils import run_bass_kernel_spmd

    assert u.shape == (B_GLOBAL, 3) and grid.shape == (GD, GH, GW, GC)
    if _nc_cache is None:
        _nc_cache = _build_nc()
    nc = _nc_cache

    q = _pack_grid(grid)
    u = np.ascontiguousarray(u, dtype=np.float32)
    in_maps = [
        {"u": u[c * B_LOCAL:(c + 1) * B_LOCAL], "q": q} for c in range(N_CORES)
    ]
    res = run_bass_kernel_spmd(nc, in_maps, core_ids=list(range(N_CORES)))
    out = np.concatenate([res.results[c]["o"] for c in range(N_CORES)], axis=0)
    return out.astype(np.float32)


if __name__ == "__main__":
    # quick self-run with random inputs
    rng = np.random.default_rng(0)
    grid = rng.standard_normal((GD, GH, GW, GC), dtype=np.float32)
    u = rng.random((B_GLOBAL, 3), dtype=np.float32)
    out = kernel(u, grid)
    print("out", out.shape, out.dtype, float(np.abs(out).mean()))


# revision 4
# speedup vs baseline: 2.2954x; 1.8412x over previous
"""Tricubic B-spline grid interpolation (CubicBSplineGrid3d) on 8 Trainium2 cores.

Strategy (data-parallel over queries, per sharding hint):
  * Host: fold the w-axis B-spline basis matrix into the packed grid
    (polynomial basis): rows keyed (d, h, sw) hold
        Q[d, h, sw, m, c, i, j] = sum_k BMAT[m,k] Gpad[d+i, h+j, sw+k, c]
    in bf16, so each query's data is ONE contiguous 4KB run and the
    on-device w-contraction is sum_m tw^m * P_m.  Queries are sorted by
    base cell per core (better HBM page locality for the gather); the
    output is unpermuted on host.
  * Device (per core, 16384 queries = 128 blocks of 128):
      stage 1: floor/frac of u*63, base indices first (gathers start
               early), then cubic basis weights for d,h axes and
               tw powers; PE-transpose to query-on-partition layout.
      main loop (engines split):
        GPSIMD: indirect DMA gathers [128, 2048] bf16 per block
        DVE:    m1 = tw*P1 (tensor_scalar, 4x mode)
        ACT:    m2 = tw^2*P2, m3 = tw^3*P3 (scale-by-partition-scalar)
        PE:     psum[blk] = P0 + m1 + m2 + m3 (identity matmuls, f32 acc)
        DVE:    per 4 blocks: multiply psum by wd_i*wh_j (ij innermost)
                and add-tree over ij; DMA out [128, 4, 32] f32.
  * Host: concat + unpermute the 8 cores' outputs.
"""
import sys

for _p in ("/opt/trn_rl_repo",):
    if _p not in sys.path:
        sys.path.insert(0, _p)

import numpy as np

N_CORES = 8
B_GLOBAL = 131072
B_LOCAL = B_GLOBAL // N_CORES          # 16384
NBLK = B_LOCAL // 128                  # 128 blocks of 128 queries
GD = GH = GW = 64                      # grid spatial dims
GC = 32                                # channels
QROWS = GD * GH * GW                   # 64*64*64 = 262144
GLEN = 4 * 4 * 4 * GC                  # 2048 elements = 4KB bf16 per query
MSLICE = 4 * 4 * GC                    # 512 elements per polynomial slice

_nc_cache = None


def _build_nc():
    """Build + compile the per-core Bass program (identical on all cores)."""
    from concourse import bacc, mybir
    from concourse.bass import IndirectOffsetOnAxis
    from concourse.tile import TileContext
    from concourse.masks import make_identity

    f32, i32 = mybir.dt.float32, mybir.dt.int32
    bf16 = mybir.dt.bfloat16
    Alu = mybir.AluOpType
    Act = mybir.ActivationFunctionType
    P = 128

    nc = bacc.Bacc("TRN2", target_bir_lowering=False, debug=False,
                   num_devices=N_CORES)
    u_t = nc.dram_tensor("u", [B_LOCAL, 3], f32, kind="ExternalInput")
    q_t = nc.dram_tensor("q", [QROWS, GLEN], bf16, kind="ExternalInput")
    o_t = nc.dram_tensor("o", [B_LOCAL, GC], f32, kind="ExternalOutput")

    with TileContext(nc) as tc:
        with (
            tc.tile_pool(name="persist", bufs=1) as pp,
            tc.tile_pool(name="stage1", bufs=1) as s1,
            tc.tile_pool(name="g", bufs=6) as gp,
            tc.tile_pool(name="m", bufs=6) as mp,
            tc.tile_pool(name="pm", bufs=2) as pmp,
            tc.tile_pool(name="tree", bufs=2) as tp_,
            tc.tile_pool(name="o", bufs=4) as op_,
        ):
            # ---------- stage 1a: indices first, so gathers start early ----
            U = s1.tile([P, 384], f32)
            nc.sync.dma_start(
                out=U[:, :], in_=u_t[:, :].rearrange("(p n) c -> p (n c)", p=P))
            X = s1.tile([P, 384], f32)
            nc.vector.tensor_scalar(X[:, :], U[:, :], float(GD - 1), None, Alu.mult)
            # floor via round-to-nearest cast + correction
            Si = s1.tile([P, 384], i32)
            nc.vector.tensor_copy(out=Si[:, :], in_=X[:, :])
            Sf = s1.tile([P, 384], f32)
            nc.vector.tensor_copy(out=Sf[:, :], in_=Si[:, :])
            D = s1.tile([P, 384], f32)
            nc.vector.tensor_tensor(out=D[:, :], in0=X[:, :], in1=Sf[:, :],
                                    op=Alu.subtract)
            M = s1.tile([P, 384], f32)
            nc.vector.tensor_scalar(M[:, :], D[:, :], 0.0, None, Alu.is_lt)
            S = s1.tile([P, 384], f32)
            nc.vector.tensor_tensor(out=S[:, :], in0=Sf[:, :], in1=M[:, :],
                                    op=Alu.subtract)
            T = s1.tile([P, 384], f32)
            nc.vector.tensor_tensor(out=T[:, :], in0=X[:, :], in1=S[:, :],
                                    op=Alu.subtract)

            S3 = S[:, :].rearrange("p (n c) -> p n c", c=3)
            # base = (sd*64 + sh)*64 + sw
            Bse = s1.tile([P, 128], f32)
            nc.vector.scalar_tensor_tensor(
                out=Bse[:, :], in0=S3[:, :, 0], scalar=float(GH),
                in1=S3[:, :, 1], op0=Alu.mult, op1=Alu.add)
            nc.vector.scalar_tensor_tensor(
                out=Bse[:, :], in0=Bse[:, :], scalar=float(GW),
                in1=S3[:, :, 2], op0=Alu.mult, op1=Alu.add)

            ident = pp.tile([P, P], f32)
            make_identity(nc, ident[:, :])
            identb = pp.tile([P, P], bf16)
            nc.vector.tensor_copy(out=identb[:, :], in_=ident[:, :])

            FB = pp.tile([P, 128], f32)   # base  [query, block]
            IdxI = pp.tile([P, 128], i32)

            with tc.tile_pool(name="psum1", bufs=2, space="PSUM") as ps1:
                def transpose_into(dst_ap, src_ap):
                    pt = ps1.tile([P, P], f32, space="PSUM")
                    nc.tensor.transpose(out=pt[:, :], in_=src_ap,
                                        identity=ident[:, :])
                    nc.vector.tensor_copy(out=dst_ap, in_=pt[:, :])

                transpose_into(FB[:, :], Bse[:, :])
                nc.vector.tensor_copy(out=IdxI[:, :], in_=FB[:, :])

                # ---------- stage 1b: weights (overlap with first gathers) --
                Tv = T[:, :].rearrange("p (n c) -> p n c", c=3)
                T2 = s1.tile([P, 384], f32)
                nc.vector.tensor_tensor(out=T2[:, :], in0=T[:, :], in1=T[:, :],
                                        op=Alu.mult)
                T3 = s1.tile([P, 384], f32)
                nc.vector.tensor_tensor(out=T3[:, :], in0=T2[:, :], in1=T[:, :],
                                        op=Alu.mult)
                T2v = T2[:, :].rearrange("p (n c) -> p n c", c=3)
                T3v = T3[:, :].rearrange("p (n c) -> p n c", c=3)

                # tw powers, transposed to [query, block]
                TW1 = pp.tile([P, 128], f32)
                TW2 = pp.tile([P, 128], f32)
                TW3 = pp.tile([P, 128], f32)
                transpose_into(TW1[:, :], Tv[:, :, 2])
                transpose_into(TW2[:, :], T2v[:, :, 2])
                transpose_into(TW3[:, :], T3v[:, :, 2])

                # cubic basis weights for d,h on [128, 384] (w cols unused)
                sixth = 1.0 / 6.0
                W0 = s1.tile([P, 384], f32)
                nc.vector.tensor_scalar(W0[:, :], T3[:, :], -sixth, None, Alu.mult)
                nc.vector.scalar_tensor_tensor(out=W0[:, :], in0=T2[:, :],
                                               scalar=0.5, in1=W0[:, :],
                                               op0=Alu.mult, op1=Alu.add)
                nc.vector.scalar_tensor_tensor(out=W0[:, :], in0=T[:, :],
                                               scalar=-0.5, in1=W0[:, :],
                                               op0=Alu.mult, op1=Alu.add)
                nc.vector.tensor_scalar(W0[:, :], W0[:, :], sixth, None, Alu.add)
                W1 = s1.tile([P, 384], f32)
                nc.vector.tensor_scalar(W1[:, :], T3[:, :], 0.5, None, Alu.mult)
                nc.vector.scalar_tensor_tensor(out=W1[:, :], in0=T2[:, :],
                                               scalar=-1.0, in1=W1[:, :],
                                               op0=Alu.mult, op1=Alu.add)
                nc.vector.tensor_scalar(W1[:, :], W1[:, :], 2.0 / 3.0, None,
                                        Alu.add)
                W3 = s1.tile([P, 384], f32)
                nc.vector.tensor_scalar(W3[:, :], T3[:, :], sixth, None, Alu.mult)
                W2 = s1.tile([P, 384], f32)
                nc.vector.tensor_tensor(out=W2[:, :], in0=W0[:, :], in1=W1[:, :],
                                        op=Alu.add)
                nc.vector.tensor_tensor(out=W2[:, :], in0=W2[:, :], in1=W3[:, :],
                                        op=Alu.add)
                nc.vector.tensor_scalar(W2[:, :], W2[:, :], -1.0, 1.0,
                                        Alu.mult, Alu.add)

                TD = pp.tile([P, 512], f32)   # wd_i  at cols i*128 + b
                TH = pp.tile([P, 512], f32)   # wh_j  at cols j*128 + b
                Ws = [W0, W1, W2, W3]
                for a, Tt in ((0, TD), (1, TH)):
                    for i in range(4):
                        w3v = Ws[i][:, :].rearrange("p (n c) -> p n c", c=3)
                        transpose_into(Tt[:, i * 128:(i + 1) * 128],
                                       w3v[:, :, a])

                # WDH2[q, b*16 + (i*4+j)] = wd_i[q,b] * wh_j[q,b]  (ij inner)
                WDH2 = pp.tile([P, 2048], f32)
                W2v = WDH2[:, :].rearrange("p (b ij) -> p b ij", ij=16)
                for i in range(4):
                    for j in range(4):
                        nc.vector.tensor_tensor(
                            out=W2v[:, :, i * 4 + j],
                            in0=TD[:, i * 128:(i + 1) * 128],
                            in1=TH[:, j * 128:(j + 1) * 128],
                            op=Alu.mult)
                WDH2b = pp.tile([P, 2048], bf16)
                nc.vector.tensor_copy(out=WDH2b[:, :], in_=WDH2[:, :])

            # ---------- main loop over 128 query blocks ----------
            # G run layout per partition: [m(4), c(32), ij(16)] bf16.
            with tc.tile_pool(name="psum2", bufs=2, space="PSUM") as ps2:
                for b in range(NBLK):
                    blk = b % 4
                    G = gp.tile([P, GLEN], bf16)
                    nc.gpsimd.indirect_dma_start(
                        out=G[:, :],
                        out_offset=None,
                        in_=q_t[:, :],
                        in_offset=IndirectOffsetOnAxis(ap=IdxI[:, b:b + 1],
                                                       axis=0),
                    )
                    # m_i = tw^i * P_i: DVE does m1 (4x TS), ACT does m2, m3
                    Mt = mp.tile([P, 3 * MSLICE], bf16)
                    nc.vector.tensor_scalar(
                        Mt[:, 0:MSLICE], G[:, MSLICE:2 * MSLICE],
                        TW1[:, b:b + 1], None, Alu.mult)
                    nc.scalar.activation(
                        out=Mt[:, MSLICE:2 * MSLICE],
                        in_=G[:, 2 * MSLICE:3 * MSLICE],
                        func=Act.Copy, scale=TW2[:, b:b + 1])
                    nc.scalar.activation(
                        out=Mt[:, 2 * MSLICE:3 * MSLICE],
                        in_=G[:, 3 * MSLICE:4 * MSLICE],
                        func=Act.Copy, scale=TW3[:, b:b + 1])
                    # PE: psum[blk] = P0 + m1 + m2 + m3
                    if blk == 0:
                        psA = ps2.tile([P, 4 * MSLICE], f32, space="PSUM")
                    sl = psA[:, blk * MSLICE:(blk + 1) * MSLICE]
                    nc.tensor.matmul(out=sl, lhsT=identb[:, :],
                                     rhs=G[:, 0:MSLICE], start=True, stop=False)
                    nc.tensor.matmul(out=sl, lhsT=identb[:, :],
                                     rhs=Mt[:, 0:MSLICE], start=False, stop=False)
                    nc.tensor.matmul(out=sl, lhsT=identb[:, :],
                                     rhs=Mt[:, MSLICE:2 * MSLICE],
                                     start=False, stop=False)
                    nc.tensor.matmul(out=sl, lhsT=identb[:, :],
                                     rhs=Mt[:, 2 * MSLICE:3 * MSLICE],
                                     start=False, stop=True)
                    if blk == 3:
                        b0 = b - 3
                        A4v = psA[:, :].rearrange("p (blk c ij) -> p blk c ij",
                                                  blk=4, ij=16)
                        wb = (WDH2b[:, b0 * 16:(b0 + 4) * 16]
                              .rearrange("p (blk x ij) -> p blk x ij",
                                         blk=4, x=1)
                              .to_broadcast([P, 4, GC, 16]))
                        Pm = pmp.tile([P, 4 * MSLICE], bf16)
                        Pm4 = Pm[:, :].rearrange("p (blk c ij) -> p blk c ij",
                                                 blk=4, ij=16)
                        nc.vector.tensor_tensor(out=Pm4[:, :, :, :],
                                                in0=A4v[:, :, :, :],
                                                in1=wb, op=Alu.mult)
                        # add-tree over ij (innermost 16 -> 1)
                        T1 = tp_.tile([P, 4, GC, 8], bf16)
                        nc.vector.tensor_tensor(out=T1[:, :, :, :],
                                                in0=Pm4[:, :, :, 0:8],
                                                in1=Pm4[:, :, :, 8:16],
                                                op=Alu.add)
                        T2t = tp_.tile([P, 4, GC, 4], bf16)
                        nc.vector.tensor_tensor(out=T2t[:, :, :, :],
                                                in0=T1[:, :, :, 0:4],
                                                in1=T1[:, :, :, 4:8], op=Alu.add)
                        T3t = tp_.tile([P, 4, GC, 2], bf16)
                        nc.vector.tensor_tensor(out=T3t[:, :, :, :],
                                                in0=T2t[:, :, :, 0:2],
                                                in1=T2t[:, :, :, 2:4],
                                                op=Alu.add)
                        o4 = op_.tile([P, 4, GC], f32)
                        nc.vector.tensor_tensor(out=o4[:, :, :],
                                                in0=T3t[:, :, :, 0],
                                                in1=T3t[:, :, :, 1], op=Alu.add)
                        nc.sync.dma_start(
                            out=o_t[b0 * 128:(b + 1) * 128, :].rearrange(
                                "(blk q) c -> q blk c", blk=4),
                            in_=o4[:, :, :])
    nc.compile()
    return nc


# Uniform cubic B-spline basis matrix: ww_k(t) = sum_m BMAT[m,k] * t^m
BMAT = np.array([
    [1.0, 4.0, 1.0, 0.0],
    [-3.0, 0.0, 3.0, 0.0],
    [3.0, -6.0, 3.0, 0.0],
    [-1.0, 3.0, -3.0, 1.0],
], dtype=np.float32) / 6.0


def _pack_grid(grid: np.ndarray) -> np.ndarray:
    """(64,64,64,32) -> [QROWS, GLEN] bf16 with
    Q[d,h,sw, m, c, i, j] = sum_k BMAT[m,k] Gpad[d+i, h+j, sw+k, c]."""
    import ml_dtypes

    gp = np.pad(np.asarray(grid, dtype=np.float32),
                ((1, 2), (1, 2), (1, 2), (0, 0)), mode="edge")
    # fold the w-axis basis first on the small padded grid:
    # PM[dp, hp, sw, m, c] = sum_k BMAT[m,k] gp[dp, hp, sw+k, c]
    gpw = np.lib.stride_tricks.sliding_window_view(gp, 4, axis=2)
    # gpw: [67, 67, 64, 32, 4]
    PM = np.einsum("mk,abwck->abwmc", BMAT, gpw, optimize=True)
    # window (i,j): Q[d,h,w, m, c, i, j] = PM[d+i, h+j, w, m, c]
    win = np.lib.stride_tricks.sliding_window_view(PM, (4, 4), axis=(0, 1))
    # win: [64, 64, 64, 4m, 32c, 4i, 4j]
    out = np.empty((QROWS, GLEN), dtype=ml_dtypes.bfloat16)
    ov = out.reshape(GD, GH, GW, 4, GC, 4, 4)
    for d0 in range(0, GD, 8):
        ov[d0:d0 + 8] = win[d0:d0 + 8].astype(ml_dtypes.bfloat16)
    return out


def _sort_perm(u_shard: np.ndarray) -> np.ndarray:
    """Sort queries by base cell for HBM page locality in the gather."""
    s = np.clip(np.floor(u_shard * (GD - 1)).astype(np.int64), 0, GD - 1)
    key = (s[:, 0] * GH + s[:, 1]) * GW + s[:, 2]
    return np.argsort(key, kind="stable")


def kernel(u: np.ndarray, grid: np.ndarray) -> np.ndarray:
    global _nc_cache
    from concourse.bass_utils import run_bass_kernel_spmd

    assert u.shape == (B_GLOBAL, 3) and grid.shape == (GD, GH, GW, GC)
    if _nc_cache is None:
        _nc_cache = _build_nc()
    nc = _nc_cache

    q = _pack_grid(grid)
    u = np.ascontiguousarray(u, dtype=np.float32)
    perms = []
    in_maps = []
    for c in range(N_CORES):
        u_shard = u[c * B_LOCAL:(c + 1) * B_LOCAL]
        perm = _sort_perm(u_shard)
        perms.append(perm)
        in_maps.append({"u": np.ascontiguousarray(u_shard[perm]), "q": q})
    res = run_bass_kernel_spmd(nc, in_maps, core_ids=list(range(N_CORES)))
    out = np.empty((B_GLOBAL, GC), dtype=np.float32)
    for c in range(N_CORES):
        blk = out[c * B_LOCAL:(c + 1) * B_LOCAL]
        blk[perms[c]] = res.results[c]["o"]
    return out


if __name__ == "__main__":
    # quick self-run with random inputs
    rng = np.random.default_rng(0)
    grid = rng.standard_normal((GD, GH, GW, GC), dtype=np.float32)
    u = rng.random((B_GLOBAL, 3), dtype=np.float32)
    out = kernel(u, grid)
    print("out", out.shape, out.dtype, float(np.abs(out).mean()))


# revision 6
# speedup vs baseline: 2.6249x; 1.1436x over previous
"""Tricubic B-spline grid interpolation (CubicBSplineGrid3d) on 8 Trainium2 cores.

Strategy (data-parallel over queries, per sharding hint):
  * Host: fold the w-axis B-spline basis matrix into the packed grid
    (polynomial basis): rows keyed (d, h, sw) hold
        Q[d, h, sw, m, c, i, j] = sum_k BMAT[m,k] Gpad[d+i, h+j, sw+k, c]
    in bf16, so each query's data is ONE contiguous 4KB run and the
    on-device w-contraction is sum_m tw^m * P_m.  Queries are sorted by
    base cell per core (better HBM page locality for the gather); the
    output is unpermuted on host.  Per-query scalar prep (base row index,
    t_w powers, wd_i*wh_j products) is tiny O(B) math done on host and
    shipped transposed, so gathers start immediately.
  * Device (per core, 16384 queries = 128 blocks of 128, engines split):
        GPSIMD: indirect DMA gathers [128, 2048] bf16 per block
        ACT:    m2 = tw^2*P2 -> PSUM slice (bank base), m3 = tw^3*P3
        DVE:    m1 = tw*P1 (tensor_scalar, 4x mode)
        PE:     psum[blk] += P0, m1, m3 (identity matmuls, f32 accumulate)
        DVE:    per 4 blocks: multiply psum by wd_i*wh_j (ij innermost)
                and add-tree over ij; DMA out [128, 4, 32] f32.
  * Host: concat + unpermute the 8 cores' outputs.
"""
import sys

for _p in ("/opt/trn_rl_repo",):
    if _p not in sys.path:
        sys.path.insert(0, _p)

import numpy as np

N_CORES = 8
B_GLOBAL = 131072
B_LOCAL = B_GLOBAL // N_CORES          # 16384
NBLK = B_LOCAL // 128                  # 128 blocks of 128 queries
GD = GH = GW = 64                      # grid spatial dims
GC = 32                                # channels
QROWS = GD * GH * GW                   # 64*64*64 = 262144
GLEN = 4 * 4 * 4 * GC                  # 2048 elements = 4KB bf16 per query
MSLICE = 4 * 4 * GC                    # 512 elements per polynomial slice

_nc_cache = None


def _build_nc():
    """Build + compile the per-core Bass program (identical on all cores)."""
    from concourse import bacc, mybir
    from concourse.bass import IndirectOffsetOnAxis
    from concourse.tile import TileContext
    from concourse.masks import make_identity

    f32, i32 = mybir.dt.float32, mybir.dt.int32
    bf16 = mybir.dt.bfloat16
    Alu = mybir.AluOpType
    Act = mybir.ActivationFunctionType
    P = 128

    nc = bacc.Bacc("TRN2", target_bir_lowering=False, debug=False,
                   num_devices=N_CORES)
    q_t = nc.dram_tensor("q", [QROWS, GLEN], bf16, kind="ExternalInput")
    bT_t = nc.dram_tensor("bT", [P, NBLK], i32, kind="ExternalInput")
    tw_t = nc.dram_tensor("twT", [P, 3 * NBLK], f32, kind="ExternalInput")
    wdh_t = nc.dram_tensor("wdhT", [P, 16 * NBLK], bf16, kind="ExternalInput")
    o_t = nc.dram_tensor("o", [B_LOCAL, GC], f32, kind="ExternalOutput")

    with TileContext(nc) as tc:
        with (
            tc.tile_pool(name="persist", bufs=1) as pp,
            tc.tile_pool(name="g", bufs=8) as gp,
            tc.tile_pool(name="m", bufs=6) as mp,
            tc.tile_pool(name="pm", bufs=2) as pmp,
            tc.tile_pool(name="tree", bufs=2) as tp_,
            tc.tile_pool(name="o", bufs=4) as op_,
            tc.tile_pool(name="psum2", bufs=2, space="PSUM") as ps2,
        ):
            # ---------- tiny prologue: load host-precomputed tables --------
            IdxI = pp.tile([P, NBLK], i32)
            nc.sync.dma_start(out=IdxI[:, :], in_=bT_t[:, :])
            TW = pp.tile([P, 3 * NBLK], f32)   # [tw | tw^2 | tw^3] per block
            nc.sync.dma_start(out=TW[:, :], in_=tw_t[:, :])
            WDHB = pp.tile([P, 16 * NBLK], bf16)
            nc.sync.dma_start(out=WDHB[:, :], in_=wdh_t[:, :])

            ident = pp.tile([P, P], f32)
            make_identity(nc, ident[:, :])
            identb = pp.tile([P, P], bf16)
            nc.vector.tensor_copy(out=identb[:, :], in_=ident[:, :])

            # ---------- main loop over 128 query blocks ----------
            # G run layout per partition: [m(4), c(32), ij(16)] bf16.
            for b in range(NBLK):
                blk = b % 4
                G = gp.tile([P, GLEN], bf16)
                nc.gpsimd.indirect_dma_start(
                    out=G[:, :],
                    out_offset=None,
                    in_=q_t[:, :],
                    in_offset=IndirectOffsetOnAxis(ap=IdxI[:, b:b + 1],
                                                   axis=0),
                )
                if blk == 0:
                    psA = ps2.tile([P, 4 * MSLICE], f32, space="PSUM")
                sl = psA[:, blk * MSLICE:(blk + 1) * MSLICE]
                # DVE: m1 = tw * P1 (4x tensor_scalar)
                Mt = mp.tile([P, 3 * MSLICE], bf16)
                nc.vector.tensor_scalar(
                    Mt[:, 0:MSLICE], G[:, MSLICE:2 * MSLICE],
                    TW[:, b:b + 1], None, Alu.mult)
                # ACT: m2 = tw^2 * P2, m3 = tw^3 * P3 (SBUF)
                nc.scalar.activation(
                    out=Mt[:, MSLICE:2 * MSLICE],
                    in_=G[:, 2 * MSLICE:3 * MSLICE],
                    func=Act.Copy, scale=TW[:, NBLK + b:NBLK + b + 1])
                nc.scalar.activation(
                    out=Mt[:, 2 * MSLICE:3 * MSLICE],
                    in_=G[:, 3 * MSLICE:4 * MSLICE],
                    func=Act.Copy, scale=TW[:, 2 * NBLK + b:2 * NBLK + b + 1])
                # PE: psum[blk] = P0 + m1 + m2 + m3
                nc.tensor.matmul(out=sl, lhsT=identb[:, :],
                                 rhs=G[:, 0:MSLICE], start=True, stop=False)
                nc.tensor.matmul(out=sl, lhsT=identb[:, :],
                                 rhs=Mt[:, 0:MSLICE], start=False, stop=False)
                nc.tensor.matmul(out=sl, lhsT=identb[:, :],
                                 rhs=Mt[:, MSLICE:2 * MSLICE],
                                 start=False, stop=False)
                nc.tensor.matmul(out=sl, lhsT=identb[:, :],
                                 rhs=Mt[:, 2 * MSLICE:3 * MSLICE],
                                 start=False, stop=True)
                if blk == 3:
                    b0 = b - 3
                    A4v = psA[:, :].rearrange("p (blk c ij) -> p blk c ij",
                                              blk=4, ij=16)
                    wb = (WDHB[:, b0 * 16:(b0 + 4) * 16]
                          .rearrange("p (blk x ij) -> p blk x ij",
                                     blk=4, x=1)
                          .to_broadcast([P, 4, GC, 16]))
                    Pm = pmp.tile([P, 4 * MSLICE], bf16)
                    Pm4 = Pm[:, :].rearrange("p (blk c ij) -> p blk c ij",
                                             blk=4, ij=16)
                    nc.vector.tensor_tensor(out=Pm4[:, :, :, :],
                                            in0=A4v[:, :, :, :],
                                            in1=wb, op=Alu.mult)
                    # add-tree over ij (innermost 16 -> 1)
                    T1 = tp_.tile([P, 4, GC, 8], bf16)
                    nc.vector.tensor_tensor(out=T1[:, :, :, :],
                                            in0=Pm4[:, :, :, 0:8],
                                            in1=Pm4[:, :, :, 8:16],
                                            op=Alu.add)
                    T2t = tp_.tile([P, 4, GC, 4], bf16)
                    nc.vector.tensor_tensor(out=T2t[:, :, :, :],
                                            in0=T1[:, :, :, 0:4],
                                            in1=T1[:, :, :, 4:8], op=Alu.add)
                    T3t = tp_.tile([P, 4, GC, 2], bf16)
                    nc.vector.tensor_tensor(out=T3t[:, :, :, :],
                                            in0=T2t[:, :, :, 0:2],
                                            in1=T2t[:, :, :, 2:4],
                                            op=Alu.add)
                    o4 = op_.tile([P, 4, GC], f32)
                    nc.vector.tensor_tensor(out=o4[:, :, :],
                                            in0=T3t[:, :, :, 0],
                                            in1=T3t[:, :, :, 1], op=Alu.add)
                    nc.sync.dma_start(
                        out=o_t[b0 * 128:(b + 1) * 128, :].rearrange(
                            "(blk q) c -> q blk c", blk=4),
                        in_=o4[:, :, :])
    nc.compile()
    return nc


# Uniform cubic B-spline basis matrix: ww_k(t) = sum_m BMAT[m,k] * t^m
BMAT = np.array([
    [1.0, 4.0, 1.0, 0.0],
    [-3.0, 0.0, 3.0, 0.0],
    [3.0, -6.0, 3.0, 0.0],
    [-1.0, 3.0, -3.0, 1.0],
], dtype=np.float32) / 6.0


def _pack_grid(grid: np.ndarray) -> np.ndarray:
    """(64,64,64,32) -> [QROWS, GLEN] bf16 with
    Q[d,h,sw, m, c, i, j] = sum_k BMAT[m,k] Gpad[d+i, h+j, sw+k, c]."""
    import ml_dtypes

    gp = np.pad(np.asarray(grid, dtype=np.float32),
                ((1, 2), (1, 2), (1, 2), (0, 0)), mode="edge")
    # fold the w-axis basis first on the small padded grid:
    # PM[dp, hp, sw, m, c] = sum_k BMAT[m,k] gp[dp, hp, sw+k, c]
    gpw = np.lib.stride_tricks.sliding_window_view(gp, 4, axis=2)
    # gpw: [67, 67, 64, 32, 4]
    PM = np.einsum("mk,abwck->abwmc", BMAT, gpw, optimize=True)
    # window (i,j): Q[d,h,w, m, c, i, j] = PM[d+i, h+j, w, m, c]
    win = np.lib.stride_tricks.sliding_window_view(PM, (4, 4), axis=(0, 1))
    # win: [64, 64, 64, 4m, 32c, 4i, 4j]
    out = np.empty((QROWS, GLEN), dtype=ml_dtypes.bfloat16)
    ov = out.reshape(GD, GH, GW, 4, GC, 4, 4)
    for d0 in range(0, GD, 8):
        ov[d0:d0 + 8] = win[d0:d0 + 8].astype(ml_dtypes.bfloat16)
    return out


def _bspline_w(t: np.ndarray) -> np.ndarray:
    """Uniform cubic B-spline weights, (n,) -> (n, 4), float32."""
    t = t.astype(np.float32)
    t2 = t * t
    t3 = t2 * t
    sixth = np.float32(1.0 / 6.0)
    w0 = (-t3 + 3.0 * t2 - 3.0 * t + 1.0) * sixth
    w1 = (3.0 * t3 - 6.0 * t2 + 4.0) * sixth
    w2 = (-3.0 * t3 + 3.0 * t2 + 3.0 * t + 1.0) * sixth
    w3 = t3 * sixth
    return np.stack([w0, w1, w2, w3], axis=-1).astype(np.float32)


def _prep_core(u_shard: np.ndarray, q: np.ndarray):
    """Per-core host prep: sort by base cell, build transposed tables."""
    import ml_dtypes

    x = u_shard.astype(np.float32) * np.float32(GD - 1)
    s = np.floor(x)
    t = x - s
    si = np.clip(s.astype(np.int64), 0, GD - 1)
    key = (si[:, 0] * GH + si[:, 1]) * GW + si[:, 2]
    perm = np.argsort(key, kind="stable")
    base = key[perm].astype(np.int32)
    t = t[perm]

    def tr(a):  # [16384] block-major -> [128 part, 128 blk]
        return np.ascontiguousarray(a.reshape(NBLK, 128).T)

    tw = t[:, 2].astype(np.float32)
    twp = np.concatenate([tr(tw), tr(tw * tw), tr(tw * tw * tw)],
                         axis=1).astype(np.float32)
    wd = _bspline_w(t[:, 0])                       # (n, 4)
    wh = _bspline_w(t[:, 1])
    wdh = (wd[:, :, None] * wh[:, None, :]).reshape(-1, 16)  # (n, 16) ij
    # [part, blk*16 + ij]
    wdhT = np.ascontiguousarray(
        wdh.reshape(NBLK, 128, 16).transpose(1, 0, 2).reshape(128, NBLK * 16)
    ).astype(ml_dtypes.bfloat16)
    return perm, {"q": q, "bT": tr(base), "twT": twp, "wdhT": wdhT}


def kernel(u: np.ndarray, grid: np.ndarray) -> np.ndarray:
    global _nc_cache
    from concourse.bass_utils import run_bass_kernel_spmd

    assert u.shape == (B_GLOBAL, 3) and grid.shape == (GD, GH, GW, GC)
    if _nc_cache is None:
        _nc_cache = _build_nc()
    nc = _nc_cache

    q = _pack_grid(grid)
    u = np.ascontiguousarray(u, dtype=np.float32)
    perms = []
    in_maps = []
    for c in range(N_CORES):
        perm, im = _prep_core(u[c * B_LOCAL:(c + 1) * B_LOCAL], q)
        perms.append(perm)
        in_maps.append(im)
    res = run_bass_kernel_spmd(nc, in_maps, core_ids=list(range(N_CORES)))
    out = np.empty((B_GLOBAL, GC), dtype=np.float32)
    for c in range(N_CORES):
        blk = out[c * B_LOCAL:(c + 1) * B_LOCAL]
        blk[perms[c]] = res.results[c]["o"]
    return out


if __name__ == "__main__":
    # quick self-run with random inputs
    rng = np.random.default_rng(0)
    grid = rng.standard_normal((GD, GH, GW, GC), dtype=np.float32)
    u = rng.random((B_GLOBAL, 3), dtype=np.float32)
    out = kernel(u, grid)
    print("out", out.shape, out.dtype, float(np.abs(out).mean()))
